# revision 1
# baseline (speedup 1.0000x reference)
"""Trainium2 Bass kernel for nn_CGRegressorAdapter (GNN message passing).

Strategy (cone-restricted):
  - The regression head only reads ONE node per graph (last_idx), so each
    layer of the 8-layer GNN stack only needs the node's influence cone:
    V_4={v} at the top, growing by in-neighborhoods down to V_{-1} (~1400
    nodes max) at the embed layer.  Host prep computes nested cone
    orderings (V_{k+1} is a prefix of V_k) and compacted adjacency slices
    M_l = A[V_{l-2}, V_{l-1}] (edge counts, exact in bf16).
  - Data-parallel over B=32 graphs: 8 cores x 4 slots.  Graphs are sorted
    by cone cost; slot j holds ranks [8j, 8j+8) and is sized to that
    quartile's EXACT per-level maxes (no 128-padding on free axes; the
    contraction runs 128-row chunks with a partial last chunk), so the
    small top layers cost almost nothing.
  - Adjacency slices ship as per-slot fp8-e4m3 blobs (edge counts <=16
    are exact) upcast to bf16 in-flight by SWDGE casting DMAs; embed
    inputs for all slots ship as one [40, sum Pm1] bf16 pack (embW rows
    >=40 are zero, so the matmul contracts 40 partitions); weights ship
    as two packed tiles.  DMA priority: embed weights, embed inputs, GNN
    weights, head weights, M blobs smallest slot first (big slots split
    so their L1 can start on the first half).
  - Per slot: embed (bf16 hi/lo one-hot matmul, f32-exact), 4 base + 4
    adapter GraphConvs, all matmuls single-bf16 (states bf16, weights
    bf16), f32 PSUM accumulate.  m-chunks are batched 4-at-a-time in one
    [128,512] PSUM tile and cast with a single DVE/ACT copy (alternating
    engines).  Both head inputs are written straight from the final
    base/adapter aggregation PSUMs as un-rounded f32 columns (no
    extraction hop, no bf16 rounding).  Measured end-to-end rel err
    6.3e-3 vs the 2e-2 gate.
  - The four slot streams are emitted in a skewed staircase (stream i
    runs i stages behind) so layer-boundary ACT waits hide under other
    slots' matmuls and PSUM agg buffers are never oversubscribed; within
    a stream, base layer i+1 is emitted before adapter layer i (they are
    independent) to shorten the drained-tail critical chain.
  - Nested prefix ordering makes the self path a plain prefix slice and
    the final extraction column 0.  Regression head (relu-free layer
    pairs constant-folded on host) on-chip in f32.
"""
import numpy as np
import ml_dtypes

import concourse.bass as bass
import concourse.mybir as mybir
from concourse import bacc
from concourse.bass import ts
from concourse.bass_utils import run_bass_kernel_spmd
from concourse.tile import TileContext

BF16 = ml_dtypes.bfloat16
FP8 = ml_dtypes.float8_e4m3
F32 = np.float32

B, N, E, H, L, VOCAB = 32, 2048, 8192, 128, 4, 32
N_CORES = 8
NG = B // N_CORES          # graphs (slots) per core
dt = mybir.dt
Alu = mybir.AluOpType
Act = mybir.ActivationFunctionType

# bias column indices in the packed bias tile
BCOL_BASE = 0      # 0..3  base_b
BCOL_ADAPT = 4     # 4..7  adapt_b
BCOL_HB1 = 8
BCOL_HMID = 9      # 9..11
BCOL_HB5 = 12
NBCOL = 16


def _ceil128(x):
    return max(128, (int(x) + 127) // 128 * 128)


def _chunks(n):
    """[(col_off, rows)] covering n in 128-row chunks, last may be partial."""
    return [(j * 128, min(128, n - j * 128)) for j in range((n + 127) // 128)]


def _spans(width, maxw=512):
    out = []
    off = 0
    while off < width:
        w = min(maxw, width - off)
        out.append((off, w))
        off += w
    return out


def _blob_layout(sizes):
    """Free-axis offsets of the per-slot bf16 blob [128, W].
    Sections: erhs [128, Pm1], then M_l as [128, (pin/128)*pout] l=1..5."""
    Pm1, P0, P1, P2, P3 = sizes
    P4 = 1
    dims = [(Pm1, P0), (P0, P1), (P1, P2), (P2, P3), (P3, P4)]
    lay = {}
    off = 0
    for l, (pin, pout) in enumerate(dims):
        w = len(_chunks(pin)) * pout
        lay[f"m{l + 1}"] = (off, w)
        off += w
    lay["_total"] = off
    lay["_dims"] = dims
    return lay


DMA_ORDER = (3, 2, 1, 0)
MP_BUFS = 8
PSUM_AGG_BUFS = 4
PSUM_M_BUFS = 4
SKEW_ORDER = (3, 2, 1, 0)
GWIDTH = 3
CAST_PAR = 0


def _build_program(slot_sizes, reps=1):
    """slot_sizes: tuple of 4 tuples (Pm1, P0, P1, P2, P3) padded sizes.
    reps>1 repeats the whole body serially (timing: slope removes
    dispatch overhead)."""
    nc = bacc.Bacc("TRN2", target_bir_lowering=False, debug=False,
                   num_devices=N_CORES)
    f32, bf16 = dt.float32, dt.bfloat16
    P4 = 1
    lays = [_blob_layout(s) for s in slot_sizes]

    # all weights packed into two tiles: bf16 (embed + GNN) and f32 (head)
    WB = 2 * H + L * 6 * H          # embw hi/lo + per layer bwn,bws,awn2,aws2
    WF = 3 * H + 1 + NBCOL          # hwa(2H) + hwb(H) + hw5(1) + biases
    wb_d = nc.declare_dram_parameter("wpack_bf", [128, WB], bf16, isOutput=False)
    wf_d = nc.declare_dram_parameter("wpack_f32", [128, WF], f32, isOutput=False)
    EP = sum(sz[0] for sz in slot_sizes)      # all slots' erhs, 40 rows
    ep_d = nc.declare_dram_parameter("epack", [40, EP], bf16, isOutput=False)
    eoffs = [sum(sz[0] for sz in slot_sizes[:s]) for s in range(NG)]
    fp8 = dt.float8e4
    blob_d = [nc.declare_dram_parameter(f"blob{s}", [128, lays[s]["_total"]],
                                        fp8, isOutput=False)
              for s in range(NG)]
    y_d = nc.declare_dram_parameter("y", [1, NG], f32, isOutput=True)

    with TileContext(nc) as tc:
        with (
            tc.tile_pool(name="const", bufs=1) as const,
            tc.tile_pool(name="state", bufs=1) as state,
            tc.tile_pool(name="mp", bufs=MP_BUFS) as mp,
            tc.tile_pool(name="psum_agg", bufs=PSUM_AGG_BUFS, space="PSUM") as psum_agg,
            tc.tile_pool(name="psum_m", bufs=PSUM_M_BUFS, space="PSUM") as psum_m,
        ):
            # ---- all input DMAs issued up front (prefetch) ----
            blob_t = [None] * NG
            ep_holder = [None]

            wb_t = const.tile([128, WB], bf16)
            wf_t = const.tile([128, WF], f32)
            consts_loaded = [False]

            def load_blobs():
                # DMA priority: embed weights, embed inputs, GNN weights,
                # head weights, then M blobs smallest slot first
                if not consts_loaded[0]:
                    nc.sync.dma_start(wb_t[:, :2 * H], wb_d[:, :2 * H])
                ep_holder[0] = state.tile([40, EP], bf16, tag="epack",
                                          name="epack")
                nc.sync.dma_start(ep_holder[0][:], ep_d[:])
                if not consts_loaded[0]:
                    nc.sync.dma_start(wb_t[:, 2 * H:], wb_d[:, 2 * H:])
                    nc.sync.dma_start(wf_t[:], wf_d[:])
                    consts_loaded[0] = True
                for s in DMA_ORDER:
                    blob_t[s] = state.tile([128, lays[s]["_total"]], bf16,
                                           tag=f"blob{s}", name=f"blob{s}")
                    half = (lays[s]["m1"][1] // 2 // 128) * 128
                    # SWDGE casting DMA: fp8 in HBM (counts are exact),
                    # bf16 in SBUF — halves the dominant DMA traffic
                    if half == 0:
                        nc.gpsimd.dma_start(blob_t[s][:], blob_d[s][:])
                    else:
                        # split so the slot's L1 can start on the first half
                        nc.gpsimd.dma_start(blob_t[s][:, :half],
                                            blob_d[s][:, :half])
                        nc.gpsimd.dma_start(blob_t[s][:, half:],
                                            blob_d[s][:, half:])
            embw_hi = wb_t[:, 0:H]
            embw_lo = wb_t[:, H:2 * H]
            bwn_t, bws_t, awn_t, aws_t = [], [], [], []
            for i in range(L):
                o = 2 * H + i * 6 * H
                bwn_t.append(wb_t[:, o:o + H])
                bws_t.append(wb_t[:, o + H:o + 2 * H])
                awn_t.append((wb_t[:, o + 2 * H:o + 3 * H],
                              wb_t[:, o + 3 * H:o + 4 * H]))
                aws_t.append((wb_t[:, o + 4 * H:o + 5 * H],
                              wb_t[:, o + 5 * H:o + 6 * H]))
            hwa0 = wf_t[:, 0:H]
            hwa1 = wf_t[:, H:2 * H]
            hwb = wf_t[:, 2 * H:3 * H]
            hw5 = wf_t[:, 3 * H:3 * H + 1]
            BOFF = 3 * H + 1

            def bias_ap(col):
                return wf_t[:, BOFF + col:BOFF + col + 1]

            gbT = state.tile([128, NG], f32, tag="gb")
            gaT = state.tile([128, NG], f32, tag="ga")

            # per-span PSUM agg tiles are fixed [128,512] and reused by tag
            def get_aggs(width):
                return [(psum_agg.tile([128, 512], f32, tag="agg0",
                                       name="agg0"), off, w)
                        for i, (off, w) in enumerate(_spans(width))]

            def gconv(blob, moff, nbr_srcs, self_srcs, p_in, p_out, bias_col,
                      out_tile, col0_out=None):
                """nbr_srcs: list of (stateT [128,p_in] bf16, [W_hi, W_lo]
                rhs aps).  self_srcs: list of (stateT, [Wself hi/lo lhsT
                aps]).  blob[:, moff+j*p_out :] holds the bf16 count slice
                for chunk j."""
                chks = _chunks(p_in)       # [(col_off, rows)], exact sizes
                nchunks = len(chks)
                aggs = get_aggs(p_out)
                nterm = sum(len(ws) for _, ws in nbr_srcs)
                GW = GWIDTH                # m chunks per grouped cast
                groups = [list(range(g, min(g + GW, nchunks)))
                          for g in range(0, nchunks, GW)]

                def emit_group(gi):
                    grp = groups[gi]
                    pm = psum_m.tile([128, 512], f32, tag="pm")
                    for jj, j in enumerate(grp):
                        co, rj = chks[j]
                        k = 0
                        for src, ws in nbr_srcs:
                            for w in ws:
                                nc.tensor.matmul(pm[:rj, jj * 128:jj * 128 + 128],
                                                 src[:, co:co + rj], w,
                                                 start=(k == 0),
                                                 stop=(k == nterm - 1))
                                k += 1
                    wd_g = len(grp) * 128
                    mhi = mp.tile([128, 512], bf16, tag="mhi")
                    if gi % 2 == CAST_PAR:
                        nc.vector.tensor_copy(out=mhi[:, :wd_g], in_=pm[:, :wd_g])
                    else:
                        nc.scalar.copy(mhi[:, :wd_g], pm[:, :wd_g])
                    return mhi

                gq = [emit_group(0)]
                # self path: bf16 weights against bf16 state
                k = 0
                for src, ws in self_srcs:
                    for w in ws:
                        for a, off, wd in aggs:
                            nc.tensor.matmul(a[:, :wd], w, src[:, off:off + wd],
                                             start=(k == 0), stop=False)
                        k += 1
                for gi, grp in enumerate(groups):
                    mhi = gq.pop(0)
                    if gi + 1 < len(groups):
                        gq.append(emit_group(gi + 1))
                    for jj, j in enumerate(grp):
                        rj = chks[j][1]
                        base = moff + j * p_out
                        for a, off, wd in aggs:
                            nc.tensor.matmul(a[:, :wd],
                                             mhi[:rj, jj * 128:jj * 128 + 128],
                                             blob[:rj, base + off:base + off + wd],
                                             start=False,
                                             stop=(j == nchunks - 1))
                for a, off, wd in aggs:
                    nc.scalar.activation(out_tile[:, off:off + wd],
                                         a[:, :wd], Act.Relu,
                                         bias=bias_ap(bias_col))
                if col0_out is not None:
                    # un-rounded f32 copy of column 0 straight from PSUM
                    # (head input) — no extraction hop off the state tile
                    nc.scalar.activation(col0_out, aggs[0][0][:, 0:1],
                                         Act.Relu, bias=bias_ap(bias_col))

            def slot_stages(s):
                """Emission closures for one slot: [embed, base1, adapt1,
                base2, ...].  Two slots are interleaved stage-by-stage so
                each layer-boundary ACT wait is hidden under the other
                slot's matmuls."""
                Pm1, P0, P1, P2, P3 = slot_sizes[s]
                lay = lays[s]
                blob = blob_t[s]
                psz = [P0, P1, P2, P3, P4]
                xT = state.tile([128, Pm1], bf16, tag=f"x{s}", name=f"x{s}")
                lat = [xT] + [state.tile([128, psz[k]], bf16, tag=f"lat{k+1}_{s}",
                                         name=f"lat{k+1}_{s}")
                              for k in range(L)]
                currs = [xT] + [state.tile([128, psz[k + 1]], bf16,
                                           tag=f"curr{k+1}_{s}",
                                           name=f"curr{k+1}_{s}")
                                for k in range(L)]
                pins = [Pm1, P0, P1, P2]
                stages = []

                def embed_stage():
                    eoff = eoffs[s]
                    ept = ep_holder[0]
                    for i_sp, (a, off, wd) in enumerate(get_aggs(Pm1)):
                        nc.tensor.matmul(a[:, :wd], embw_hi[:40, :],
                                         ept[:, eoff + off:eoff + off + wd],
                                         start=True, stop=False)
                        nc.tensor.matmul(a[:, :wd], embw_lo[:40, :],
                                         ept[:, eoff + off:eoff + off + wd],
                                         start=False, stop=True)
                        if i_sp % 2 == 0:
                            nc.vector.tensor_copy(out=xT[:, off:off + wd],
                                                  in_=a[:, :wd])
                        else:
                            nc.scalar.copy(xT[:, off:off + wd], a[:, :wd])
                stages.append(embed_stage)

                def base_stage(i):
                    def run():
                        gconv(blob, lay[f"m{i+1}"][0],
                              nbr_srcs=[(lat[i], [bwn_t[i]])],
                              self_srcs=[(lat[i], [bws_t[i]])],
                              p_in=pins[i], p_out=psz[i],
                              bias_col=BCOL_BASE + i, out_tile=lat[i + 1],
                              col0_out=(gbT[:, s:s + 1] if i == L - 1
                                        else None))
                    return run

                def adapt_stage(i):
                    def run():
                        # the last adapter output is only read at column 0
                        # (the head input): write it straight into gaT and
                        # skip the extraction hop on the critical tail
                        out_t = currs[i + 1] if i < L - 1 else gaT[:, s:s + 1]
                        gconv(blob, lay[f"m{i+2}"][0],
                              nbr_srcs=[(lat[i + 1], [awn_t[i][0]]),
                                        (currs[i], [awn_t[i][1]])],
                              self_srcs=[(lat[i + 1], [aws_t[i][0]]),
                                         (currs[i], [aws_t[i][1]])],
                              p_in=psz[i], p_out=psz[i + 1],
                              bias_col=BCOL_ADAPT + i, out_tile=out_t)
                    return run

                # base_{i+1} ahead of adapt_i: they are independent, so in
                # the drained tail the base chain advances while the adapter
                # fills its ACT waits (critical depth ~6 instead of 8)
                stages.append(base_stage(0))
                for i in range(L - 1):
                    stages.append(base_stage(i + 1))
                    stages.append(adapt_stage(i))
                stages.append(adapt_stage(L - 1))
                return stages


            # ---- regression head (all slots at once) ----
            def whole_pass():
                load_blobs()
                streams = [slot_stages(ss) for ss in SKEW_ORDER]
                nst = len(streams[0])
                for r in range(nst + len(streams) - 1):
                    for i, stream in enumerate(streams):
                        k = r - i
                        if 0 <= k < nst:
                            stream[k]()
                emit_head()

            def head_mm(lhsT, rhs, bias_col, func):
                pm = psum_m.tile([128, 128], f32, tag="pm")
                nc.tensor.matmul(pm[:, :NG], lhsT, rhs, start=True, stop=True)
                out = state.tile([128, NG], f32, tag="hy")
                nc.scalar.activation(out[:], pm[:, :NG], func,
                                     bias=bias_ap(bias_col))
                return out

            def emit_head():
                # head with relu-free pairs constant-folded on host:
                # y = ((relu(g@Wa+ba))@Wb+bb -> relu) @ hW5 + hb5
                pm = psum_m.tile([128, 128], f32, tag="pm")
                nc.tensor.matmul(pm[:, :NG], hwa0, gbT[:],
                                 start=True, stop=False)
                nc.tensor.matmul(pm[:, :NG], hwa1, gaT[:],
                                 start=False, stop=True)
                y1 = state.tile([128, NG], f32, tag="hy")
                nc.scalar.activation(y1[:], pm[:, :NG], Act.Relu,
                                     bias=bias_ap(BCOL_HB1))
                y2 = head_mm(hwb, y1[:], BCOL_HMID + 0, Act.Relu)
                pm5 = psum_m.tile([128, 128], f32, tag="pm")
                nc.tensor.matmul(pm5[:1, :NG], hw5, y2[:],
                                 start=True, stop=True)
                yout = state.tile([1, NG], f32, tag="yout")
                nc.scalar.activation(yout[:], pm5[:1, :NG], Act.Identity,
                                     bias=bias_ap(BCOL_HB5)[:1])
                nc.sync.dma_start(y_d[:], yout[:])

            for _rep in range(reps):
                whole_pass()

    nc.compile()
    return nc


_NC_CACHE = {}
_LAST = {}


def _get_program(reps=1):
    key = (_LAST["slot_sizes"], reps)
    if key not in _NC_CACHE:
        _NC_CACHE[key] = _build_program(_LAST["slot_sizes"], reps=reps)
    return _NC_CACHE[key]


def _cones(edge, last_idx):
    """Nested cone ordering per graph.  Returns (order, sizes[n4..nm1])."""
    out = []
    for g in range(B):
        src, dst = edge[g, 0], edge[g, 1]
        order = [int(last_idx[g])]
        inset = np.zeros(N, bool)
        inset[order[0]] = True
        sizes = [1]
        for _ in range(5):
            new = np.unique(src[inset[dst]])
            new = new[~inset[new]]
            order.extend(new.tolist())
            inset[new] = True
            sizes.append(len(order))
        out.append((np.asarray(order), sizes))
    return out


def _split_hilo(a):
    hi = a.astype(BF16)
    lo = (a - hi.astype(F32)).astype(BF16)
    return hi, lo


def _prep_inputs(inputs):
    """Host-side cone construction + sharding.  Returns list of in_maps."""
    inds = np.asarray(inputs["regular_node_inds"]).astype(np.int64)
    shapes = np.asarray(inputs["regular_node_shapes"], dtype=F32)
    edge = np.asarray(inputs["edge_index"]).astype(np.int64)
    last_idx = np.asarray(inputs["last_idx"]).astype(np.int64)

    cones = _cones(edge, last_idx)
    # sort graphs by cost; slot j <- ranks [8j, 8j+8), core c <- rank 8j+c
    cost = np.array([c[1][5] + c[1][4] for c in cones])
    ranks = np.argsort(-cost, kind="stable")
    assign = ranks.reshape(NG, N_CORES)          # [slot, core] -> graph id
    slot_sizes = []
    for s in range(NG):
        gs = assign[s]
        mx = [max(cones[g][1][k] for g in gs) for k in range(6)]
        # sizes[k] = |V_{4-k}|; exact per-level maxes (Pm1,P0,P1,P2,P3)
        slot_sizes.append(tuple(int(mx[5 - l]) for l in range(5)))
    slot_sizes = tuple(slot_sizes)
    _LAST["slot_sizes"] = slot_sizes
    _LAST["assign"] = assign
    lays = [_blob_layout(s) for s in slot_sizes]

    # embed weights, hi/lo bf16 pair (exact): rows 0..31 table, 32..35 and
    # 36..39 shape_w (paired against shapes_hi / shapes_lo blob rows)
    embed_w = np.zeros((128, H), dtype=F32)
    embed_w[:VOCAB] = np.asarray(inputs["embed_table"], dtype=F32)
    embed_w[VOCAB:VOCAB + 4] = np.asarray(inputs["shape_w"], dtype=F32)
    embed_w[VOCAB + 4:VOCAB + 8] = np.asarray(inputs["shape_w"], dtype=F32)
    ehi, elo = _split_hilo(embed_w)
    # the shape_w rows must stay IDENTICAL in both copies within each of
    # hi/lo (they are), pairing: x = oh@(thi+tlo) + (shi+slo)@(swhi+swlo)
    embed_w2 = np.stack([ehi, elo], axis=1)     # [128, 2, H]

    bws2 = np.asarray(inputs["base_Wself"], dtype=F32).astype(BF16)
    bwn2 = np.asarray(inputs["base_Wnbr"], dtype=F32).astype(BF16)
    aws = np.asarray(inputs["adapt_Wself"], dtype=F32).reshape(L, 2, H, H)
    awn = np.asarray(inputs["adapt_Wnbr"], dtype=F32).reshape(L, 2, H, H)
    aws2 = np.ascontiguousarray(aws.transpose(0, 2, 1, 3)).astype(BF16)
    awn2 = np.ascontiguousarray(awn.transpose(0, 2, 1, 3)).astype(BF16)
    hW1 = np.asarray(inputs["hW1"], np.float64)
    hb1 = np.asarray(inputs["hb1"], np.float64)
    hWm = np.asarray(inputs["hWmid"], np.float64)
    hbm = np.asarray(inputs["hbmid"], np.float64)
    Wa = hW1 @ hWm[0]                       # [2H, H]
    ba = hb1 @ hWm[0] + hbm[0]
    Wb = hWm[1] @ hWm[2]                    # [H, H]
    bb = hbm[1] @ hWm[2] + hbm[2]
    hw1 = np.ascontiguousarray(
        Wa.astype(F32).reshape(2, H, H).transpose(1, 0, 2))

    biases = np.zeros((H, NBCOL), dtype=F32)
    biases[:, BCOL_BASE:BCOL_BASE + L] = np.asarray(inputs["base_b"], dtype=F32).T
    biases[:, BCOL_ADAPT:BCOL_ADAPT + L] = np.asarray(inputs["adapt_b"], dtype=F32).T
    biases[:, BCOL_HB1] = ba.astype(F32)
    biases[:, BCOL_HMID] = bb.astype(F32)
    biases[0, BCOL_HB5] = np.asarray(inputs["hb5"], dtype=F32)[0]

    WB = 2 * H + L * 6 * H
    WF = 3 * H + 1 + NBCOL
    wpack_bf = np.zeros((128, WB), dtype=BF16)
    wpack_bf[:, 0:H] = embed_w2[:, 0, :]
    wpack_bf[:, H:2 * H] = embed_w2[:, 1, :]
    for i in range(L):
        o = 2 * H + i * 6 * H
        wpack_bf[:, o:o + H] = bwn2[i]
        wpack_bf[:, o + H:o + 2 * H] = bws2[i]
        wpack_bf[:, o + 2 * H:o + 3 * H] = awn2[i][:, 0, :]
        wpack_bf[:, o + 3 * H:o + 4 * H] = awn2[i][:, 1, :]
        wpack_bf[:, o + 4 * H:o + 5 * H] = aws2[i][:, 0, :]
        wpack_bf[:, o + 5 * H:o + 6 * H] = aws2[i][:, 1, :]
    wpack_f32 = np.zeros((128, WF), dtype=F32)
    wpack_f32[:, 0:H] = Wa.astype(F32)[:H, :]
    wpack_f32[:, H:2 * H] = Wa.astype(F32)[H:, :]
    wpack_f32[:, 2 * H:3 * H] = Wb.astype(F32)
    wpack_f32[:, 3 * H:3 * H + 1] = np.asarray(inputs["hW5"], dtype=F32)
    wpack_f32[:, 3 * H + 1:] = biases
    shared = {"wpack_bf": wpack_bf, "wpack_f32": wpack_f32}
    in_maps = [dict(shared) for _ in range(N_CORES)]
    EP = sum(sz[0] for sz in slot_sizes)
    epack = [np.zeros((40, EP), dtype=BF16) for _ in range(N_CORES)]
    for s in range(NG):
        Pm1, P0, P1, P2, P3 = slot_sizes[s]
        lay = lays[s]
        for c in range(N_CORES):
            g = assign[s, c]
            order, sizes = cones[g]
            n = len(order)
            pos = np.full(N, -1, np.int64)
            pos[order] = np.arange(n)
            src, dst = edge[g, 0], edge[g, 1]
            ps, pd = pos[src], pos[dst]
            blob = np.zeros((128, lay["_total"]), dtype=FP8)
            # erhs: one-hot rows 0..31, shapes hi rows 32..35, lo rows 36..39
            eoff = sum(sz[0] for sz in slot_sizes[:s])
            erhs = np.zeros((40, Pm1), dtype=F32)
            erhs[inds[g][order], np.arange(n)] = 1.0
            shi, slo = _split_hilo(shapes[g][order].T)
            epack[c][:, eoff:eoff + Pm1] = erhs.astype(BF16)
            epack[c][VOCAB:VOCAB + 4, eoff:eoff + n] = shi[:, :n]
            epack[c][VOCAB + 4:VOCAB + 8, eoff:eoff + n] = slo[:, :n]
            for l, (pin, pout) in enumerate(lay["_dims"]):
                ncols = sizes[4 - l]   # |V_{l-1}|
                rceil = ((pin + 127) // 128) * 128
                M = np.zeros((rceil, pout), dtype=F32)
                mask = (pd >= 0) & (pd < ncols)
                np.add.at(M, (ps[mask], pd[mask]), 1.0)
                moff = lay[f"m{l + 1}"][0]
                # chunk-major on the free axis, stride pout, exact widths
                assert M.max() <= 16, "edge multiplicity exceeds fp8-exact range"
                Mt = M.astype(FP8).reshape(rceil // 128, 128, pout)
                blob[:, moff:moff + (rceil // 128) * pout] = (
                    Mt.transpose(1, 0, 2).reshape(128, -1))
            in_maps[c][f"blob{s}"] = blob
    for c in range(N_CORES):
        in_maps[c]["epack"] = epack[c]
    return in_maps


def kernel(**inputs) -> np.ndarray:
    in_maps = _prep_inputs(inputs)
    nc = _get_program()
    assign = _LAST["assign"]
    # first dispatch after a fresh compile has produced garbage before
    # (axon staging race); run twice and keep the steady-state result
    run_bass_kernel_spmd(nc, in_maps, core_ids=list(range(N_CORES)))
    res = run_bass_kernel_spmd(nc, in_maps, core_ids=list(range(N_CORES)))
    out = np.zeros((B, 1), dtype=F32)
    for c in range(N_CORES):
        yc = np.asarray(res.results[c]["y"]).reshape(NG)
        for s in range(NG):
            out[assign[s, c], 0] = yc[s]
    return out



# revision 8
# speedup vs baseline: 2.3624x; 2.3624x over previous
"""Trainium2 Bass kernel for nn_CGRegressorAdapter (GNN message passing).

Design (combined-axis + structural input pre-aggregation):
  - Only one node per graph feeds the head (last_idx), so each GNN layer
    needs just the root's influence cone: A_4={root} .. A_0 (~500 nodes).
    Host computes nested cone orderings (A_{l+1} is a prefix of A_l) per
    graph; each of the 8 cores processes 4 graphs laid out on ONE combined
    node axis per level (slot blocks at shared bases, per-slot zero pads),
    so every engine instruction covers 4 graphs at once.
  - Layer 1 collapses entirely: the host pre-aggregates the *structural*
    inputs over edges into V_0 (vocab one-hot counts C and raw shape sums
    SH, hi/lo bf16) and the embed weights are folded with the layer-1
    weights on host (embW@W, bf16).  agg_L1 = foldedW.T @ [C;SH] is ONE
    full-span matmul; the self path streams the node features the same
    way.  No big M1 adjacency, no L1 message/cast pipeline at all.
  - Layers 2..8 (3 base + 4 adapter): per 128-chunk of the in-axis,
    messages = state_chunk.T @ W into PSUM (transpose fused), cast to
    bf16 (DVE/ACT alternating), then aggregated against banded adjacency
    blocks: within each level the extension nodes are ordered by their
    first out-neighbor position, so each (chunk x slot) adjacency block
    only spans a narrow column window (host computes exact windows,
    unioned across cores).  Blocks ship as one fp8 blob (counts <=16 are
    exact) upcast to bf16 in flight by a SWDGE casting DMA.
  - PSUM bracketing: hardware clears the whole 2KB zero-region's
    has_written bits on a start=True matmul, so per span the FIRST matmul
    is full-width (the L1 fold term, or a [1xH] zeros x ones matmul) and
    everything after accumulates sub-ranges; biases ride the ACT read.
  - Two reps are software-pipelined at emission (rep r's first half
    zipped with rep r-1's second half) so layer-boundary ACT latency is
    hidden by independent work; all tiles live in bufs=2 pools so DMA and
    compute of consecutive reps overlap fully.
  - Head: constant-folded on host to 3 matmuls on [128,4] f32 columns
    extracted straight from the final base/adapter PSUMs (strided AP for
    the 4 roots).  Measured end-to-end rel err ~7e-3 vs the 2e-2 gate.
"""
import numpy as np
import ml_dtypes

import concourse.mybir as mybir
from concourse import bacc
from concourse.bass_utils import run_bass_kernel_spmd
from concourse.tile import TileContext

BF16 = ml_dtypes.bfloat16
FP8 = ml_dtypes.float8_e4m3
F32 = np.float32

B, N, E, H, L, VOCAB = 32, 2048, 8192, 128, 4, 32
N_CORES = 8
NG = B // N_CORES          # graphs (slots) per core
NLEV = 5                   # axes A_0..A_4
dt = mybir.dt
Act = mybir.ActivationFunctionType

GW = 4                     # msg chunks per [128,512] PSUM group
MP_BUFS = 6
PSUM_AGG_BUFS = 5
PSUM_M_BUFS = 3

# bias column indices in the f32 bias pack (ACT bias reads)
BCOL_BASE = 0      # 0..3  base_b
BCOL_ADAPT = 4     # 4..7  adapt_b
BCOL_HB1 = 8
BCOL_HMID = 9
BCOL_HB5 = 10
NBCOL = 12

# wrow (row-0 bf16 pack): 8 bias rows (base1..4, adapt1..4) + ones
ONES_OFF = 8 * H
WROW = ONES_OFF + 512


def _spans(width, maxw=512):
    out = []
    off = 0
    while off < width:
        w = min(maxw, width - off)
        out.append((off, w))
        off += w
    return out


def _cones_g(edge, root):
    """Nested cone orders [A_0, A_1, A_2, A_3, A_4]; A_{l+1} prefix of A_l.
    Extension nodes sorted by their min out-neighbor position (banding)."""
    src, dst = edge
    order = np.array([root], np.int64)
    orders = [order]
    inset = np.zeros(N, bool)
    inset[root] = True
    pos = np.full(N, -1, np.int64)
    pos[root] = 0
    for _ in range(4):
        m = inset[dst]
        key = np.full(N, 1 << 60, np.int64)
        np.minimum.at(key, src[m], pos[dst[m]])
        new = np.unique(src[m])
        new = new[~inset[new]]
        new = new[np.argsort(key[new], kind="stable")]
        order = np.concatenate([order, new])
        inset[new] = True
        pos[order] = np.arange(len(order))
        orders.append(order)
    return orders[::-1]


def _split_hilo(a):
    hi = np.asarray(a, F32).astype(BF16)
    lo = (np.asarray(a, F32) - hi.astype(F32)).astype(BF16)
    return hi, lo


_NC_CACHE = {}
_LAST = {}


def _prep_inputs(inputs):
    inds = np.asarray(inputs["regular_node_inds"]).astype(np.int64)
    shapes = np.asarray(inputs["regular_node_shapes"], dtype=F32)
    edge = np.asarray(inputs["edge_index"]).astype(np.int64)
    last_idx = np.asarray(inputs["last_idx"]).astype(np.int64)

    cones = [_cones_g(edge[g], int(last_idx[g])) for g in range(B)]
    cost = np.array([len(c[0]) + len(c[1]) for c in cones])
    ranks = np.argsort(-cost, kind="stable")
    # snake assignment: balances per-core combined level sizes
    assign = np.zeros((NG, N_CORES), np.int64)
    for j in range(NG):
        row = ranks[8 * j:8 * j + 8]
        assign[j] = row if j % 2 == 0 else row[::-1]
    _LAST["assign"] = assign

    # padded per-slot sizes & bases per level (shared across cores)
    SZ = np.zeros((NLEV, NG), np.int64)
    for l in range(NLEV):
        for s in range(NG):
            SZ[l, s] = max(len(cones[assign[s, c]][l]) for c in range(N_CORES))
    S3u = int(SZ[3].max())
    SZ[3, :] = S3u
    SZ[4, :] = 1
    base = np.zeros((NLEV, NG), np.int64)
    for l in range(NLEV):
        base[l] = np.cumsum(SZ[l]) - SZ[l]
    Pt = tuple(int(SZ[l].sum()) for l in range(NLEV))

    # adjacency blocks, unioned across cores: blocks[k] for edges into V_k
    blocks = {}
    counts = {}   # (k, c) -> list of (rows, cols, vals) in combined coords
    for k in range(1, 5):
        lohi = {}
        for c in range(N_CORES):
            rows_all, cols_all = [], []
            for s in range(NG):
                g = assign[s, c]
                oin, oout = cones[g][k - 1], cones[g][k]
                pin = np.full(N, -1, np.int64)
                pin[oin] = base[k - 1, s] + np.arange(len(oin))
                pout = np.full(N, -1, np.int64)
                pout[oout] = base[k, s] + np.arange(len(oout))
                m = pout[edge[g, 1]] >= 0
                r = pin[edge[g, 0][m]]
                q = pout[edge[g, 1][m]]
                assert (r >= 0).all()
                rows_all.append(r)
                cols_all.append(q)
            rows = np.concatenate(rows_all)
            cols = np.concatenate(cols_all)
            counts[(k, c)] = (rows, cols)
            for j in np.unique(rows // 128):
                m = rows // 128 == j
                # split by slot on the column side
                cj = cols[m]
                for s in range(NG):
                    ms = (cj >= base[k, s]) & (cj < base[k, s] + SZ[k, s])
                    if not ms.any():
                        continue
                    lo, hi = int(cj[ms].min()), int(cj[ms].max()) + 1
                    key = (int(j), s)
                    if key in lohi:
                        plo, phi = lohi[key]
                        lohi[key] = (min(lo, plo), max(hi, phi))
                    else:
                        lohi[key] = (lo, hi)
        blk = []
        for (j, s) in sorted(lohi):
            lo, hi = lohi[(j, s)]
            blk.append((j, lo, hi - lo))
        blocks[k] = blk

    # blob layout: m2 first (needed earliest), then m3..m5
    off = 0
    blob_lay = {}
    for k in range(1, 5):
        lay = []
        for (j, lo, w) in blocks[k]:
            lay.append((j, lo, w, off))
            off += w
        blob_lay[k] = tuple(lay)
    WM = max(off, 512)
    WM_SPLIT = blob_lay[2][0][3] if blob_lay.get(2) else WM  # end of m2 region

    # self-path slices per layer k: (ibase, obase, width)
    selfs = {}
    for k in range(1, 5):
        selfs[k] = tuple((int(base[k - 1, s]), int(base[k, s]), int(SZ[k, s]))
                         for s in range(NG))

    P0t, P1t = Pt[0], Pt[1]
    assert P1t <= 512 and Pt[2] <= 512 and Pt[3] <= 512, Pt
    WA = 2 * P0t + P1t

    layout = {
        "Pt": Pt, "S3u": S3u, "selfs": selfs,
        "blob": {k: blob_lay[k] for k in range(1, 5)},
        "WM": WM, "WM_SPLIT": WM_SPLIT, "WA": WA,
    }
    key = (Pt, S3u, tuple(sorted((k, v) for k, v in layout["blob"].items())),
           tuple(sorted(selfs.items())), WM, WM_SPLIT, WA)
    _LAST["layout"] = layout
    _LAST["key"] = key

    # ---- weights (shared across cores) ----
    T = np.asarray(inputs["embed_table"], dtype=np.float64)
    S = np.asarray(inputs["shape_w"], dtype=np.float64)
    TS = np.concatenate([T, S], axis=0)            # [36, H]
    bWs = np.asarray(inputs["base_Wself"], dtype=np.float64)
    bWn = np.asarray(inputs["base_Wnbr"], dtype=np.float64)
    aWs = np.asarray(inputs["adapt_Wself"], dtype=np.float64)
    aWn = np.asarray(inputs["adapt_Wnbr"], dtype=np.float64)

    def fold(W):
        f = TS @ W                                  # [36, H]
        out = np.zeros((128, H), F32)
        out[:32] = f[:32]
        out[32:36] = f[32:36]
        out[36:40] = f[32:36]
        return out.astype(BF16)

    WB = 4 * H + 3 * 6 * H
    wb = np.zeros((128, WB), dtype=BF16)
    wb[:, 0:H] = fold(bWn[0])
    wb[:, H:2 * H] = fold(bWs[0])
    wb[:, 2 * H:3 * H] = fold(aWn[0][H:])           # x half of adapt Wnbr
    wb[:, 3 * H:4 * H] = fold(aWs[0][H:])           # x half of adapt Wself
    for i in range(1, L):
        o = 4 * H + (i - 1) * 6 * H
        wb[:, o:o + H] = bWn[i].astype(BF16)
        wb[:, o + H:o + 2 * H] = bWs[i].astype(BF16)
        wb[:, o + 2 * H:o + 3 * H] = aWn[i][:H].astype(BF16)
        wb[:, o + 3 * H:o + 4 * H] = aWn[i][H:].astype(BF16)
        wb[:, o + 4 * H:o + 5 * H] = aWs[i][:H].astype(BF16)
        wb[:, o + 5 * H:o + 6 * H] = aWs[i][H:].astype(BF16)
    # adapter L1 lat half weights (aWn[0][:H], aWs[0][:H])
    WB2 = WB + 2 * H
    wb2 = np.zeros((128, WB2), dtype=BF16)
    wb2[:, :WB] = wb
    wb2[:, WB:WB + H] = aWn[0][:H].astype(BF16)
    wb2[:, WB + H:WB + 2 * H] = aWs[0][:H].astype(BF16)

    # head constant folding
    hW1 = np.asarray(inputs["hW1"], np.float64)
    hb1 = np.asarray(inputs["hb1"], np.float64)
    hWm = np.asarray(inputs["hWmid"], np.float64)
    hbm = np.asarray(inputs["hbmid"], np.float64)
    Wa = hW1 @ hWm[0]
    ba = hb1 @ hWm[0] + hbm[0]
    Wb = hWm[1] @ hWm[2]
    bb = hbm[1] @ hWm[2] + hbm[2]

    WF = 3 * H + 1 + NBCOL
    wf = np.zeros((128, WF), dtype=F32)
    wf[:, 0:H] = Wa.astype(F32)[:H]
    wf[:, H:2 * H] = Wa.astype(F32)[H:]
    wf[:, 2 * H:3 * H] = Wb.astype(F32)
    wf[:, 3 * H:3 * H + 1] = np.asarray(inputs["hW5"], dtype=F32)
    BOFF = 3 * H + 1
    wf[:, BOFF + BCOL_BASE:BOFF + BCOL_BASE + L] = \
        np.asarray(inputs["base_b"], dtype=F32).T
    wf[:, BOFF + BCOL_ADAPT:BOFF + BCOL_ADAPT + L] = \
        np.asarray(inputs["adapt_b"], dtype=F32).T
    wf[:, BOFF + BCOL_HB1] = ba.astype(F32)
    wf[:, BOFF + BCOL_HMID] = bb.astype(F32)
    wf[0, BOFF + BCOL_HB5] = np.asarray(inputs["hb5"], dtype=F32)[0]

    # bias rows stay ZERO: the bias x ones matmul is purely the PSUM
    # start bracket; the real bias is applied exactly by the ACT read.
    wrow = np.zeros((1, WROW), dtype=BF16)
    wrow[0, ONES_OFF:] = BF16(1.0)

    shared = {"wpack_bf": wb2, "wpack_f32": wf, "wrow": wrow}
    in_maps = [dict(shared) for _ in range(N_CORES)]

    # ---- per-core data: apack + blob ----
    for c in range(N_CORES):
        apack = np.zeros((40, WA), dtype=F32)
        for s in range(NG):
            g = assign[s, c]
            o0 = cones[g][0]
            n0 = len(o0)
            b0 = base[0, s]
            # ept features of V_0 nodes
            apack[inds[g][o0], b0 + np.arange(n0)] = 1.0
            shi, slo = _split_hilo(shapes[g][o0].T)
            apack[32:36, b0:b0 + n0] = shi.astype(F32)
            apack[36:40, b0:b0 + n0] = slo.astype(F32)
        # aggpack1: edges into V_0, cols on A_0, at offset P0t
        # aggpack2: edges into V_1, cols on A_1, at offset 2*P0t
        for s in range(NG):
            g = assign[s, c]
            src, dst = edge[g, 0], edge[g, 1]
            for (lev, coff) in ((0, P0t), (1, 2 * P0t)):
                oout = cones[g][lev]
                pout = np.full(N, -1, np.int64)
                pout[oout] = base[lev, s] + np.arange(len(oout))
                m = pout[dst] >= 0
                u, q = src[m], coff + pout[dst[m]]
                np.add.at(apack, (inds[g][u], q), 1.0)
                for kdim in range(4):
                    np.add.at(apack[32 + kdim], q, shapes[g][u, kdim])
        # hi/lo split of aggregated shape rows (rows 32:36 hold f32 sums
        # for BOTH the ept region (already hi) and agg regions).  Redo
        # cleanly: recompute hi/lo for the agg regions only.
        agg_sh = apack[32:36, P0t:].copy()
        shi, slo = _split_hilo(agg_sh)
        apack[32:36, P0t:] = shi.astype(F32)
        apack[36:40, P0t:] = slo.astype(F32)
        in_maps[c]["apack"] = apack.astype(BF16)

        blob = np.zeros((128, WM), dtype=FP8)
        for k in range(1, 5):
            rows, cols = counts[(k, c)]
            M = np.zeros((((int(rows.max()) // 128 + 1) if len(rows) else 1)
                          * 128, Pt[k]), dtype=F32)
            np.add.at(M, (rows, cols), 1.0)
            assert M.max() <= 16, "edge multiplicity exceeds fp8-exact range"
            for (j, lo, w, o) in _LAST["layout"]["blob"][k]:
                if j * 128 < M.shape[0]:
                    blk = M[j * 128:(j + 1) * 128, lo:lo + w]
                    blob[:blk.shape[0], o:o + w] = blk.astype(FP8)
        in_maps[c]["mblob"] = blob
    return in_maps


def _build_program(key, layout, reps=1):
    Pt = layout["Pt"]
    S3u = layout["S3u"]
    selfs = layout["selfs"]
    blob_lay = layout["blob"]
    WM, WM_SPLIT, WA = layout["WM"], layout["WM_SPLIT"], layout["WA"]
    P0t, P1t, P2t, P3t, P4t = Pt
    f32, bf16 = dt.float32, dt.bfloat16
    WB2 = 4 * H + 3 * 6 * H + 2 * H
    WF = 3 * H + 1 + NBCOL
    BOFF = 3 * H + 1

    nc = bacc.Bacc("TRN2", target_bir_lowering=False, debug=False,
                   num_devices=N_CORES)
    wb_d = nc.declare_dram_parameter("wpack_bf", [128, WB2], bf16, isOutput=False)
    wf_d = nc.declare_dram_parameter("wpack_f32", [128, WF], f32, isOutput=False)
    wr_d = nc.declare_dram_parameter("wrow", [1, WROW], bf16, isOutput=False)
    ap_d = nc.declare_dram_parameter("apack", [40, WA], bf16, isOutput=False)
    mb_d = nc.declare_dram_parameter("mblob", [128, WM], dt.float8e4, isOutput=False)
    y_d = nc.declare_dram_parameter("y", [1, NG], f32, isOutput=True)

    with TileContext(nc) as tc:
        with (
            tc.tile_pool(name="const", bufs=1) as const,
            tc.tile_pool(name="state", bufs=2) as state,
            tc.tile_pool(name="mp", bufs=MP_BUFS) as mp,
            tc.tile_pool(name="psum_agg", bufs=PSUM_AGG_BUFS, space="PSUM") as psum_agg,
            tc.tile_pool(name="psum_m", bufs=PSUM_M_BUFS, space="PSUM") as psum_m,
        ):
            wb_t = const.tile([128, WB2], bf16)
            wf_t = const.tile([128, WF], f32)
            wr_t = const.tile([1, WROW], bf16)
            consts_loaded = [False]

            fT_bwn = wb_t[:40, 0:H]
            fT_bws = wb_t[:40, H:2 * H]
            fT_awn = wb_t[:40, 2 * H:3 * H]
            fT_aws = wb_t[:40, 3 * H:4 * H]

            def wtile(i, which):
                o = 4 * H + (i - 1) * 6 * H
                idx = {"bwn": 0, "bws": 1, "awn_hi": 2, "awn_lo": 3,
                       "aws_hi": 4, "aws_lo": 5}[which]
                return wb_t[:, o + idx * H:o + (idx + 1) * H]

            awn0_lat = wb_t[:, WB2 - 2 * H:WB2 - H]
            aws0_lat = wb_t[:, WB2 - H:WB2]
            hwa0 = wf_t[:, 0:H]
            hwa1 = wf_t[:, H:2 * H]
            hwb = wf_t[:, 2 * H:3 * H]
            hw5 = wf_t[:, 3 * H:3 * H + 1]

            def bias_ap(col):
                return wf_t[:, BOFF + col:BOFF + col + 1]

            def brow(i):       # bias row for bias x ones matmul
                return wr_t[0:1, i * H:(i + 1) * H]

            def ones_ap(wd):
                return wr_t[0:1, ONES_OFF:ONES_OFF + wd]

            cast_par = [0]

            def cast(dst, src):
                if cast_par[0] % 2 == 0:
                    nc.vector.tensor_copy(out=dst, in_=src)
                else:
                    nc.scalar.copy(dst, src)
                cast_par[0] += 1

            def emit_msgs(srcs_ws, Pin):
                """srcs_ws: list of (state_ap, w_ap).  Returns list of
                (mhi_tile, jj, rj) indexed by chunk."""
                nchunks = (Pin + 127) // 128
                out = []
                for g0 in range(0, nchunks, GW):
                    grp = list(range(g0, min(g0 + GW, nchunks)))
                    pm = psum_m.tile([128, 512], f32, tag="pm", name="pm")
                    for jj, j in enumerate(grp):
                        co = 128 * j
                        rj = min(128, Pin - co)
                        nt = len(srcs_ws)
                        for t, (srca, w) in enumerate(srcs_ws):
                            nc.tensor.matmul(
                                pm[:rj, jj * 128:jj * 128 + 128],
                                srca[:, co:co + rj], w,
                                start=(t == 0), stop=(t == nt - 1),
                                skip_group_check=True)
                    mhi = mp.tile([128, 512], bf16, tag="mhi", name="mhi")
                    wd = len(grp) * 128
                    cast(mhi[:, :wd], pm[:, :wd])
                    for jj, j in enumerate(grp):
                        out.append((mhi, jj, min(128, Pin - 128 * j)))
                return out

            def emit_agg(k, Pout, start_fn, self_list, mhis, blob_t,
                         bias_col, out_writes):
                """One agg span (Pout <= 512).  start_fn emits the
                full-width start matmul(s); self_list: (w_ap, src_ap,
                ibase, obase, wd); blocks from blob_lay[k]; out_writes:
                list of (out_ap, in_slice_fn) act emissions."""
                a = psum_agg.tile([128, 512], f32, tag="agg", name="agg")
                blks = blob_lay[k]
                last = len(blks) + len(self_list)
                cnt = [0]

                def fl():
                    cnt[0] += 1
                    return cnt[0] == last

                start_fn(a, Pout)
                for (w, srca, ib, ob, wd) in self_list:
                    nc.tensor.matmul(a[:, ob:ob + wd], w, srca[:, ib:ib + wd],
                                     start=False, stop=fl(),
                                     skip_group_check=True)
                for (j, lo, w, o) in blks:
                    mhi, jj, rj = mhis[j]
                    nc.tensor.matmul(a[:, lo:lo + w],
                                     mhi[:rj, jj * 128:jj * 128 + 128],
                                     blob_t[:rj, o:o + w],
                                     start=False, stop=fl(),
                                     skip_group_check=True)
                for (out_ap, in_fn, func) in out_writes:
                    nc.scalar.activation(out_ap, in_fn(a), func,
                                         bias=bias_ap(bias_col))
                return a

            def whole_pass_stages(rep):
                """Returns the list of stage closures for one rep."""
                st = {}

                def dmas():
                    if not consts_loaded[0]:
                        nc.sync.dma_start(wb_t[:], wb_d[:])
                        nc.sync.dma_start(wf_t[:], wf_d[:])
                        nc.sync.dma_start(wr_t[:], wr_d[:])
                        consts_loaded[0] = True
                    st["apk"] = state.tile([40, WA], bf16, tag="apack",
                                           name="apack")
                    nc.sync.dma_start(st["apk"][:], ap_d[:])
                    st["blob"] = state.tile([128, WM], bf16, tag="blob",
                                            name="blob")
                    if 0 < WM_SPLIT < WM:
                        nc.gpsimd.dma_start(st["blob"][:, :WM_SPLIT],
                                            mb_d[:, :WM_SPLIT])
                        nc.gpsimd.dma_start(st["blob"][:, WM_SPLIT:],
                                            mb_d[:, WM_SPLIT:])
                    else:
                        nc.gpsimd.dma_start(st["blob"][:], mb_d[:])

                def l1b():
                    apk = st["apk"]
                    ept = apk[:, 0:P0t]
                    ag1 = apk[:, P0t:2 * P0t]
                    lat1 = state.tile([128, P0t], bf16, tag="lat1", name="lat1")
                    st["lat1"] = lat1
                    for i_sp, (off, wd) in enumerate(_spans(P0t)):
                        a = psum_agg.tile([128, 512], f32, tag="agg", name="agg")
                        nc.tensor.matmul(a[:, :wd], fT_bwn,
                                         ag1[:, off:off + wd],
                                         start=True, stop=False,
                                         skip_group_check=True)
                        nc.tensor.matmul(a[:, :wd], fT_bws,
                                         ept[:, off:off + wd],
                                         start=False, stop=True,
                                         skip_group_check=True)
                        nc.scalar.activation(lat1[:, off:off + wd], a[:, :wd],
                                             Act.Relu, bias=bias_ap(BCOL_BASE))

                def bias_start(i):
                    def fn(a, wd):
                        nc.tensor.matmul(a[:, :wd], brow(i), ones_ap(wd),
                                         start=True, stop=False,
                                         skip_group_check=True)
                    return fn

                def l2b():
                    lat1 = st["lat1"]
                    mhis = emit_msgs([(lat1, wtile(1, "bwn"))], P0t)
                    lat2 = state.tile([128, P1t], bf16, tag="lat2", name="lat2")
                    st["lat2"] = lat2
                    sl = [(wtile(1, "bws"), lat1, ib, ob, wd)
                          for (ib, ob, wd) in selfs[1]]
                    emit_agg(1, P1t, bias_start(1), sl, mhis, st["blob"],
                             BCOL_BASE + 1,
                             [(lat2[:], lambda a: a[:, :P1t], Act.Relu)])

                def l1a():
                    apk = st["apk"]
                    lat1 = st["lat1"]
                    ept = apk[:, 0:P0t]
                    ag2 = apk[:, 2 * P0t:2 * P0t + P1t]
                    mhis = emit_msgs([(lat1, awn0_lat)], P0t)
                    cur1 = state.tile([128, P1t], bf16, tag="cur1", name="cur1")
                    st["cur1"] = cur1

                    def start_fn(a, wd):
                        nc.tensor.matmul(a[:, :wd], fT_awn, ag2[:, :wd],
                                         start=True, stop=False,
                                         skip_group_check=True)
                    sl = []
                    for (ib, ob, wd) in selfs[1]:
                        sl.append((aws0_lat, lat1, ib, ob, wd))
                        sl.append((fT_aws, ept, ib, ob, wd))
                    emit_agg(1, P1t, start_fn, sl, mhis, st["blob"],
                             BCOL_ADAPT,
                             [(cur1[:], lambda a: a[:, :P1t], Act.Relu)])

                def base_layer(i, k):   # i = layer idx (2,3), k = i
                    def run():
                        lat = st[f"lat{i}"]
                        mhis = emit_msgs([(lat, wtile(i, "bwn"))], Pt[k - 1])
                        out = state.tile([128, Pt[k]], bf16, tag=f"lat{i+1}",
                                         name=f"lat{i+1}")
                        st[f"lat{i+1}"] = out
                        sl = [(wtile(i, "bws"), lat, ib, ob, wd)
                              for (ib, ob, wd) in selfs[k]]
                        writes = [(out[:], lambda a: a[:, :Pt[k]], Act.Relu)]
                        if i == 3:      # L4b: also extract roots -> gbT
                            gbT = state.tile([128, NG], f32, tag="gb", name="gb")
                            st["gb"] = gbT
                            writes.append(
                                (gbT[:, 0:NG],
                                 lambda a: a[:, 0:NG * S3u:S3u], Act.Relu))
                        emit_agg(k, Pt[k], bias_start(i), sl, mhis, st["blob"],
                                 BCOL_BASE + i, writes)
                    return run

                def adapt_layer(i, k):  # adapter layer i (1,2,3), k = i+1
                    def run():
                        lat = st[f"lat{i+1}"]
                        cur = st[f"cur{i}"]
                        mhis = emit_msgs([(lat, wtile(i, "awn_hi")),
                                          (cur, wtile(i, "awn_lo"))], Pt[k - 1])
                        sl = []
                        for (ib, ob, wd) in selfs[k]:
                            sl.append((wtile(i, "aws_hi"), lat, ib, ob, wd))
                            sl.append((wtile(i, "aws_lo"), cur, ib, ob, wd))
                        if i < L - 1:
                            out = state.tile([128, Pt[k]], bf16,
                                             tag=f"cur{i+1}", name=f"cur{i+1}")
                            st[f"cur{i+1}"] = out
                            writes = [(out[:], lambda a: a[:, :Pt[k]],
                                       Act.Relu)]
                        else:           # L4a -> gaT f32 directly
                            gaT = state.tile([128, NG], f32, tag="ga", name="ga")
                            st["ga"] = gaT
                            writes = [(gaT[:, 0:NG], lambda a: a[:, 0:NG],
                                       Act.Relu)]
                        emit_agg(k, Pt[k], bias_start(L + i), sl, mhis,
                                 st["blob"], BCOL_ADAPT + i, writes)
                    return run

                def head():
                    gbT, gaT = st["gb"], st["ga"]
                    pm = psum_m.tile([128, 512], f32, tag="pm", name="pm")
                    nc.tensor.matmul(pm[:, :NG], hwa0, gbT[:, :NG],
                                     start=True, stop=False,
                                     skip_group_check=True)
                    nc.tensor.matmul(pm[:, :NG], hwa1, gaT[:, :NG],
                                     start=False, stop=True,
                                     skip_group_check=True)
                    y1 = state.tile([128, NG], f32, tag="hy", name="hy")
                    nc.scalar.activation(y1[:], pm[:, :NG], Act.Relu,
                                         bias=bias_ap(BCOL_HB1))
                    pm2 = psum_m.tile([128, 512], f32, tag="pm", name="pm")
                    nc.tensor.matmul(pm2[:, :NG], hwb, y1[:, :NG],
                                     start=True, stop=True,
                                     skip_group_check=True)
                    y2 = state.tile([128, NG], f32, tag="hy2", name="hy2")
                    nc.scalar.activation(y2[:], pm2[:, :NG], Act.Relu,
                                         bias=bias_ap(BCOL_HMID))
                    pm5 = psum_m.tile([128, 512], f32, tag="pm", name="pm")
                    nc.tensor.matmul(pm5[:1, :NG], hw5, y2[:, :NG],
                                     start=True, stop=True,
                                     skip_group_check=True)
                    yout = state.tile([1, NG], f32, tag="yout", name="yout")
                    nc.scalar.activation(yout[:], pm5[:1, :NG], Act.Identity,
                                         bias=bias_ap(BCOL_HB5)[:1])
                    nc.sync.dma_start(y_d[:], yout[:])

                return [dmas, l1b, l2b, l1a, base_layer(2, 2),
                        adapt_layer(1, 2), base_layer(3, 3),
                        adapt_layer(2, 3), adapt_layer(3, 4), head]

            # software-pipeline two reps: rep r's first half zipped with
            # rep r-1's second half (independent work fills ACT latency)
            HALF = 5
            prev = None
            for r in range(reps):
                cur = whole_pass_stages(r)
                a_part = cur[:HALF]
                b_part = prev[HALF:] if prev is not None else []
                n = max(len(a_part), len(b_part))
                for i in range(n):
                    if i < len(a_part):
                        a_part[i]()
                    if i < len(b_part):
                        b_part[i]()
                prev = cur
            for fn in prev[HALF:]:
                fn()

    nc.compile()
    return nc


def _get_program(reps=1):
    key = (_LAST["key"], reps)
    if key not in _NC_CACHE:
        _NC_CACHE[key] = _build_program(_LAST["key"], _LAST["layout"], reps=reps)
    return _NC_CACHE[key]


def kernel(**inputs) -> np.ndarray:
    in_maps = _prep_inputs(inputs)
    nc = _get_program()
    assign = _LAST["assign"]
    # first dispatch after a fresh compile has produced garbage before
    # (axon staging race); run twice and keep the steady-state result
    run_bass_kernel_spmd(nc, in_maps, core_ids=list(range(N_CORES)))
    res = run_bass_kernel_spmd(nc, in_maps, core_ids=list(range(N_CORES)))
    out = np.zeros((B, 1), dtype=F32)
    for c in range(N_CORES):
        yc = np.asarray(res.results[c]["y"]).reshape(NG)
        for s in range(NG):
            out[assign[s, c], 0] = yc[s]
    return out
